# revision 11
# baseline (speedup 1.0000x reference)
"""Bahdanau attention kernel for Trainium2, data-parallel over batch on 8 cores.

Per core (B_local=2, S=8192, H=256), fp32 I/O with fp32r matmuls:
  1. Cache encoder_outputs in SBUF as f32r (cast during DMA), layout
     [128, 64, 256] with s = p*64 + t (contiguous 64KB per partition).
  2. Per 512-s score tile: PE-transpose enc -> encT [h, s], h2 = W2 @ encT
     (PSUM accum over h chunks), tanh(h2 + h1) on ScalarE (bias = h1 col),
     scores = V . tanh via PE with the 1-column V stationary.
  3. exp(scores) written s-ordered into a [1, 8192] row via a strided
     ScalarE output AP (max subtraction is skipped: |score| <= sum|V| ~ 8,
     safely inside fp32 exp range), then departitioned to [128, 64] by a
     DRAM round-trip (contiguous descriptors both ways).
  4. Normalization folded into the epilogue: Z from the transposed exps,
     context = (exp @ enc) * (1/Z) via 64 accumulating matmuls + scaled copy.
  5. context_tiled = broadcast of context over S: one [128, 512] seed per
     h-chunk, DMAed with a repeat-source access pattern (16x) to HBM.
"""

import numpy as np
from contextlib import ExitStack

import concourse.bass as bass
import concourse.bacc as bacc
import concourse.tile as tile
import concourse.bass_isa as bass_isa
import concourse.bass_utils as bass_utils
from concourse import mybir
from concourse.masks import make_identity

f32 = mybir.dt.float32
f32r = mybir.dt.float32r
AF = mybir.ActivationFunctionType
AX = mybir.AxisListType

B, S, H = 16, 8192, 256
NCORES = 8
BL = B // NCORES          # 2 batches per core
P = 128
NT = S // P               # 64 natural tiles per batch (t axis), s = p*64 + t
ST = 512                  # s elements per score tile
NST = S // ST             # 16 score tiles per batch
SUB = ST // P             # 4 psum sub-tiles per score tile
HC = H // P               # 2 h chunks
SEEDW = 512               # broadcast seed width

_CACHE = {}


def _build():
    nc = bacc.Bacc("TRN2", target_bir_lowering=False, debug=False)
    hidden = nc.dram_tensor("hidden", [BL, H], f32, kind="ExternalInput").ap()
    enc = nc.dram_tensor("enc", [BL, S, H], f32, kind="ExternalInput").ap()
    W1 = nc.dram_tensor("W1", [H, H], f32, kind="ExternalInput").ap()
    W2 = nc.dram_tensor("W2", [H, H], f32, kind="ExternalInput").ap()
    V = nc.dram_tensor("V", [1, H], f32, kind="ExternalInput").ap()
    ctx_tiled = nc.dram_tensor("ctx_tiled", [BL, H, S], f32, kind="ExternalOutput").ap()
    scr = [nc.dram_tensor(f"scr{b}", [1, S], f32, kind="ExternalOutput").ap()
           for b in range(BL)]
    ctx_out = nc.dram_tensor("ctx", [BL, H], f32, kind="ExternalOutput").ap()
    dbg = nc.dram_tensor("dbg", [1, 64], f32, kind="ExternalOutput").ap()
    attn_out = nc.dram_tensor("attn", [BL, S], f32, kind="ExternalOutput").ap()

    with tile.TileContext(nc) as tc, ExitStack() as ctx:
        singles = ctx.enter_context(tc.tile_pool(name="singles", bufs=1))
        encp = ctx.enter_context(tc.tile_pool(name="encp", bufs=1))
        work = ctx.enter_context(tc.tile_pool(name="work", bufs=2))
        pet = ctx.enter_context(tc.tile_pool(name="pet", bufs=3, space="PSUM"))
        ph2 = ctx.enter_context(tc.tile_pool(name="ph2", bufs=2, space="PSUM"))
        psm = ctx.enter_context(tc.tile_pool(name="psm", bufs=2, space="PSUM"))

        # ---- encoder cache: leading chunk, then identity, then the rest,
        # so the gpsimd engine frees the identity ops early ----
        encr = enc.rearrange("b (p t) h -> b p t h", t=NT)
        enc_sb = [encp.tile([P, NT, H], f32r, name=f"enc_sb{b}") for b in range(BL)]
        nc.gpsimd.dma_start(out=enc_sb[0][:, 0:4, :], in_=encr[0, :, 0:4, :])

        ident = singles.tile([P, P], f32, name="ident")
        make_identity(nc, ident)
        identr = singles.tile([P, P], f32r, name="identr")
        nc.vector.tensor_copy(identr, ident)
        ones11 = singles.tile([1, 1], f32, name="ones11")
        nc.vector.memset(ones11, 1.0)
        warm = singles.tile([P, P], f32r, name="warm")
        nc.vector.tensor_copy(warm, identr)
        wmps = psm.tile([P, P], f32r, name="wmps", tag="sps")
        for _ in range(30):
            nc.tensor.transpose(wmps, warm, identr)
        wmout = singles.tile([1, 4], f32, name="wmout")
        nc.vector.tensor_copy(wmout, wmps[0:1, 0:4].bitcast(f32))
        nc.gpsimd.dma_start(out=dbg[0:1, 0:4], in_=wmout)

        chunks = [(4, 16), (16, 32), (32, 48), (48, 64)]
        for b in range(BL):
            for k0, k1 in ([(0, 4)] if b > 0 else []) + chunks:
                nc.gpsimd.dma_start(out=enc_sb[b][:, k0:k1, :],
                                    in_=encr[b, :, k0:k1, :])

        # ---- weights setup: transposes via PE ----
        w2n = singles.tile([P, HC, H], f32, name="w2n")
        nc.sync.dma_start(out=w2n, in_=W2.rearrange("(kc kp) h -> kp kc h", kp=P))
        w2T = singles.tile([P, HC, H], f32r, name="w2T")
        w1n = singles.tile([P, HC, H], f32, name="w1n")
        nc.sync.dma_start(out=w1n, in_=W1.rearrange("(kc kp) h -> kp kc h", kp=P))
        w1T = singles.tile([P, HC, H], f32, name="w1T")
        for kc in range(HC):
            for hc in range(HC):
                wps = psm.tile([P, P], f32, name="wps", tag="sps")
                nc.tensor.transpose(wps, w2n[:, kc, hc * P:(hc + 1) * P], ident)
                nc.vector.tensor_copy(w2T[:, hc, kc * P:(kc + 1) * P], wps)
                wps2 = psm.tile([P, P], f32, name="wps2", tag="sps")
                nc.tensor.transpose(wps2, w1n[:, kc, hc * P:(hc + 1) * P], ident)
                nc.vector.tensor_copy(w1T[:, hc, kc * P:(kc + 1) * P], wps2)

        v_sb = singles.tile([1, H], f32, name="v_sb")
        nc.sync.dma_start(out=v_sb, in_=V)
        vT = singles.tile([P, HC], f32r, name="vT")
        hid_sb = singles.tile([BL, H], f32, name="hid_sb")
        nc.sync.dma_start(out=hid_sb, in_=hidden)
        hidT = singles.tile([P, HC, BL], f32, name="hidT")
        for c in range(HC):
            vps = psm.tile([P, 1], f32, name="vps", tag="sps")
            nc.tensor.matmul(vps, v_sb[0:1, c * P:(c + 1) * P], ones11[:], start=True, stop=True)
            nc.vector.tensor_copy(vT[:, c:c + 1], vps)
            hps = psm.tile([P, BL], f32, name="hps", tag="sps")
            nc.tensor.matmul(hps, hid_sb[:, c * P:(c + 1) * P], ident[0:BL, 0:BL], start=True, stop=True)
            nc.vector.tensor_copy(hidT[:, c, :], hps)

        # h1T[k, b] = sum_h W1[k, h] * hidden[b, h], laid out [kp, kh, b]
        h1T = singles.tile([P, HC, BL], f32, name="h1T")
        for kh in range(HC):
            h1ps = psm.tile([P, BL], f32, name="h1ps", tag="sps")
            for c in range(HC):
                nc.tensor.matmul(h1ps, w1T[:, c, kh * P:(kh + 1) * P], hidT[:, c, :],
                                 start=(c == 0), stop=(c == HC - 1))
            nc.vector.tensor_copy(h1T[:, kh, :], h1ps)

        ctx_tr = ctx_tiled.rearrange("b (c p) s -> b c p s", p=P)
        attn_r = attn_out.rearrange("b (p t) -> b p t", t=NT)

        # s-ordered exp row, shared across batches (WAR dep on the bounce DMA)
        exp_row = singles.tile([1, S], f32, name="exp_row")
        # view [o, it, j, p]: position = p*64 + it*4 + j  (= s)
        exp_rv = exp_row.rearrange("o (p i j) -> o i j p", i=NST, j=SUB)

        for b in range(BL):
            # ---- score phase ----
            for it in range(NST):
                encT = []
                for c in range(HC):
                    eps = pet.tile([P, ST], f32r, name="eps", tag="eps")
                    for j in range(SUB):
                        t = it * SUB + j
                        nc.tensor.transpose(eps[:, j * P:(j + 1) * P],
                                            enc_sb[b][:, t, c * P:(c + 1) * P], identr)
                    et = work.tile([P, ST], f32r, name=f"encT{c}", tag=f"encT{c}", bufs=3)
                    nc.vector.tensor_copy(et, eps)
                    encT.append(et)
                th = []
                for kh in range(HC):
                    h2ps = ph2.tile([P, ST], f32, name="h2ps", tag="h2ps")
                    for c in range(HC):
                        nc.tensor.matmul(h2ps, w2T[:, c, kh * P:(kh + 1) * P], encT[c],
                                         start=(c == 0), stop=(c == HC - 1))
                    tt = work.tile([P, ST], f32r, name=f"th{kh}", tag=f"th{kh}", bufs=3)
                    nc.scalar.activation(out=tt, in_=h2ps, func=AF.Tanh,
                                         bias=h1T[:, kh, b:b + 1], scale=1.0)
                    th.append(tt)
                sps = psm.tile([1, ST], f32, name="sps", tag="sps")
                for kh in range(HC):
                    nc.tensor.matmul(sps, vT[:, kh:kh + 1], th[kh],
                                     start=(kh == 0), stop=(kh == HC - 1))
                # exp (no max subtraction needed: |score| <= sum|V_h| ~ 8,
                # well inside fp32 exp range); s-ordered scatter on gpsimd
                srowE = work.tile([1, ST], f32, name="srowE", tag="srowE", bufs=2)
                nc.scalar.activation(out=srowE, in_=sps, func=AF.Exp)
                nc.gpsimd.tensor_copy(out=exp_rv[0:1, it],
                                      in_=srowE.rearrange("o (j p) -> o j p", j=SUB))

            # ---- departition via DRAM bounce (contiguous both ways) ----
            nc.sync.dma_start(out=scr[b], in_=exp_row)
            if b == BL - 1:
                # paced PE activity across the bounce stall so HAM stays at
                # 8/8 for the context matmuls (a >3.4us idle would re-throttle)
                junk = singles.tile([P, P], f32r, name="junk")
                nc.vector.tensor_copy(junk, identr)
                for w in range(24):
                    dps = psm.tile([P, P], f32r, name="dps", tag="sps")
                    nc.tensor.transpose(dps, junk, identr)
                    nc.vector.tensor_copy(junk, dps)
                jout = work.tile([1, 4], f32, name="jout", tag="jout")
                nc.vector.tensor_copy(jout, junk[0:1, 0:4].bitcast(f32))
                nc.gpsimd.dma_start(out=dbg[0:1, 8:12], in_=jout)
            expT = work.tile([P, NT], f32r, name="expT", tag=f"expT{b}", bufs=1)
            # SWDGE cast f32 -> f32r on the way back in
            nc.gpsimd.dma_start(out=expT, in_=scr[b].rearrange("o (p t) -> (o p) t", t=NT))

            # ---- normalization terms ----
            sm = work.tile([P, 1], f32, name="sm", tag="sm")
            nc.vector.reduce_sum(sm, expT.bitcast(f32), axis=AX.X)
            gsm = work.tile([P, 1], f32, name="gsm", tag="gsm")
            nc.gpsimd.partition_all_reduce(gsm, sm, channels=P, reduce_op=bass_isa.ReduceOp.add)
            inv = work.tile([P, 1], f32, name="inv", tag="inv")
            nc.vector.reciprocal(inv, gsm)
            attnN = work.tile([P, NT], f32, name="attnN", tag=f"attnN{b}", bufs=1)
            nc.vector.tensor_scalar_mul(attnN, expT.bitcast(f32), inv[:, 0:1])
            nc.gpsimd.dma_start(out=attn_r[b], in_=attnN)

            # ---- context = (exp @ enc) / Z ----
            cps = psm.tile([1, H], f32, name="cps", tag="cps", bufs=1)
            for t in range(NT):
                nc.tensor.matmul(cps, expT[:, t:t + 1], enc_sb[b][:, t, :],
                                 start=(t == 0), stop=(t == NT - 1))
            ctx_row = work.tile([1, H], f32, name="ctx_row", tag="ctx_row")
            nc.scalar.activation(out=ctx_row, in_=cps, func=AF.Copy,
                                 scale=inv[0:1, 0:1])
            nc.gpsimd.dma_start(out=ctx_out[b:b + 1, :], in_=ctx_row)
            ctxT = work.tile([P, HC], f32, name="ctxT", tag="ctxT")
            for c in range(HC):
                tps = psm.tile([P, 1], f32, name="tps", tag="sps")
                nc.tensor.matmul(tps, ctx_row[0:1, c * P:(c + 1) * P], ones11[:],
                                 start=True, stop=True)
                nc.vector.tensor_copy(ctxT[:, c:c + 1], tps)

            # ---- context_tiled broadcast: seed + repeat-source DMA ----
            if b == 0:
                # w2n is dead after setup; reuse it as the zero source
                zeros = w2n.rearrange("p a b -> p (a b)")[:, 0:SEEDW]
                nc.vector.memset(zeros, 0.0)
            for c in range(HC):
                seed = work.tile([P, SEEDW], f32, name="seed", tag="seed")
                nc.vector.tensor_scalar_add(seed, zeros, ctxT[:, c:c + 1])
                rep = bass.AP(tensor=seed.tensor, offset=seed.offset,
                              ap=[seed.ap[0], [0, S // SEEDW], seed.ap[1]])
                nc.sync.dma_start(
                    out=ctx_tr[b, c].rearrange("p (r w) -> p r w", w=SEEDW),
                    in_=rep)

    nc.compile()
    return nc


def _get_nc():
    if "nc" not in _CACHE:
        _CACHE["nc"] = _build()
    return _CACHE["nc"]


def kernel(hidden, encoder_outputs, W1, W2, V):
    hidden = np.ascontiguousarray(np.asarray(hidden, dtype=np.float32))
    enc = np.ascontiguousarray(np.asarray(encoder_outputs, dtype=np.float32))
    W1 = np.ascontiguousarray(np.asarray(W1, dtype=np.float32))
    W2 = np.ascontiguousarray(np.asarray(W2, dtype=np.float32))
    V = np.ascontiguousarray(np.asarray(V, dtype=np.float32)).reshape(1, H)

    nc = _get_nc()
    in_maps = []
    for i in range(NCORES):
        sl = slice(i * BL, (i + 1) * BL)
        in_maps.append({
            "hidden": hidden[sl],
            "enc": enc[sl],
            "W1": W1,
            "W2": W2,
            "V": V,
        })
    res = bass_utils.run_bass_kernel_spmd(nc, in_maps, core_ids=list(range(NCORES)))
    ctx_tiled = np.concatenate([r["ctx_tiled"] for r in res.results], axis=0)
    ctx = np.concatenate([r["ctx"] for r in res.results], axis=0)
    attn = np.concatenate([r["attn"] for r in res.results], axis=0)
    return ctx_tiled, ctx, attn


# revision 12
# speedup vs baseline: 1.0096x; 1.0096x over previous
"""Bahdanau attention kernel for Trainium2, data-parallel over batch on 8 cores.

Per core (B_local=2, S=8192, H=256), fp32 I/O with fp32r matmuls:
  1. Cache encoder_outputs in SBUF as f32r (cast during DMA), layout
     [128, 64, 256] with s = p*64 + t (contiguous 64KB per partition).
  2. Per 512-s score tile: PE-transpose enc -> encT [h, s], h2 = W2 @ encT
     (PSUM accum over h chunks), tanh(h2 + h1) on ScalarE (bias = h1 col),
     scores = V . tanh via PE with the 1-column V stationary.
  3. exp(scores) written s-ordered into a [1, 8192] row via a strided
     ScalarE output AP (max subtraction is skipped: |score| <= sum|V| ~ 8,
     safely inside fp32 exp range), then departitioned to [128, 64] by a
     DRAM round-trip (contiguous descriptors both ways).
  4. Normalization folded into the epilogue: Z from the transposed exps,
     context = (exp @ enc) * (1/Z) via 64 accumulating matmuls + scaled copy.
  5. context_tiled = broadcast of context over S: one [128, 512] seed per
     h-chunk, DMAed with a repeat-source access pattern (16x) to HBM.
"""

import numpy as np
from contextlib import ExitStack

import concourse.bass as bass
import concourse.bacc as bacc
import concourse.tile as tile
import concourse.bass_isa as bass_isa
import concourse.bass_utils as bass_utils
from concourse import mybir
from concourse.masks import make_identity

f32 = mybir.dt.float32
f32r = mybir.dt.float32r
AF = mybir.ActivationFunctionType
AX = mybir.AxisListType

B, S, H = 16, 8192, 256
NCORES = 8
BL = B // NCORES          # 2 batches per core
P = 128
NT = S // P               # 64 natural tiles per batch (t axis), s = p*64 + t
ST = 512                  # s elements per score tile
NST = S // ST             # 16 score tiles per batch
SUB = ST // P             # 4 psum sub-tiles per score tile
HC = H // P               # 2 h chunks
SEEDW = 512               # broadcast seed width

_CACHE = {}


def _build():
    nc = bacc.Bacc("TRN2", target_bir_lowering=False, debug=False)
    hidden = nc.dram_tensor("hidden", [BL, H], f32, kind="ExternalInput").ap()
    enc = nc.dram_tensor("enc", [BL, S, H], f32, kind="ExternalInput").ap()
    W1 = nc.dram_tensor("W1", [H, H], f32, kind="ExternalInput").ap()
    W2 = nc.dram_tensor("W2", [H, H], f32, kind="ExternalInput").ap()
    V = nc.dram_tensor("V", [1, H], f32, kind="ExternalInput").ap()
    ctx_tiled = nc.dram_tensor("ctx_tiled", [BL, H, S], f32, kind="ExternalOutput").ap()
    scr = [nc.dram_tensor(f"scr{b}", [1, S], f32, kind="ExternalOutput").ap()
           for b in range(BL)]
    ctx_out = nc.dram_tensor("ctx", [BL, H], f32, kind="ExternalOutput").ap()
    dbg = nc.dram_tensor("dbg", [1, 64], f32, kind="ExternalOutput").ap()
    attn_out = nc.dram_tensor("attn", [BL, S], f32, kind="ExternalOutput").ap()

    with tile.TileContext(nc) as tc, ExitStack() as ctx:
        singles = ctx.enter_context(tc.tile_pool(name="singles", bufs=1))
        encp = ctx.enter_context(tc.tile_pool(name="encp", bufs=1))
        work = ctx.enter_context(tc.tile_pool(name="work", bufs=2))
        pet = ctx.enter_context(tc.tile_pool(name="pet", bufs=3, space="PSUM"))
        ph2 = ctx.enter_context(tc.tile_pool(name="ph2", bufs=2, space="PSUM"))
        psm = ctx.enter_context(tc.tile_pool(name="psm", bufs=2, space="PSUM"))

        # ---- encoder cache: leading chunk, then identity, then the rest,
        # so the gpsimd engine frees the identity ops early ----
        encr = enc.rearrange("b (p t) h -> b p t h", t=NT)
        enc_sb = [encp.tile([P, NT, H], f32r, name=f"enc_sb{b}") for b in range(BL)]
        nc.gpsimd.dma_start(out=enc_sb[0][:, 0:4, :], in_=encr[0, :, 0:4, :])

        ident = singles.tile([P, P], f32, name="ident")
        make_identity(nc, ident)
        identr = singles.tile([P, P], f32r, name="identr")
        nc.vector.tensor_copy(identr, ident)
        ones11 = singles.tile([1, 1], f32, name="ones11")
        nc.vector.memset(ones11, 1.0)
        warm = singles.tile([P, P], f32r, name="warm")
        nc.vector.tensor_copy(warm, identr)
        wmps = psm.tile([P, P], f32r, name="wmps", tag="sps")
        for _ in range(30):
            nc.tensor.transpose(wmps, warm, identr)
        wmout = singles.tile([1, 4], f32, name="wmout")
        nc.vector.tensor_copy(wmout, wmps[0:1, 0:4].bitcast(f32))
        nc.gpsimd.dma_start(out=dbg[0:1, 0:4], in_=wmout)

        chunks = [(4, 16), (16, 32), (32, 48), (48, 64)]
        for b in range(BL):
            for k0, k1 in ([(0, 4)] if b > 0 else []) + chunks:
                nc.gpsimd.dma_start(out=enc_sb[b][:, k0:k1, :],
                                    in_=encr[b, :, k0:k1, :])

        # ---- weights setup: transposes via PE ----
        w2n = singles.tile([P, HC, H], f32, name="w2n")
        nc.sync.dma_start(out=w2n, in_=W2.rearrange("(kc kp) h -> kp kc h", kp=P))
        w2T = singles.tile([P, HC, H], f32r, name="w2T")
        w1n = singles.tile([P, HC, H], f32, name="w1n")
        nc.sync.dma_start(out=w1n, in_=W1.rearrange("(kc kp) h -> kp kc h", kp=P))
        w1T = singles.tile([P, HC, H], f32, name="w1T")
        for kc in range(HC):
            for hc in range(HC):
                wps = psm.tile([P, P], f32, name="wps", tag="sps")
                nc.tensor.transpose(wps, w2n[:, kc, hc * P:(hc + 1) * P], ident)
                nc.vector.tensor_copy(w2T[:, hc, kc * P:(kc + 1) * P], wps)
                wps2 = psm.tile([P, P], f32, name="wps2", tag="sps")
                nc.tensor.transpose(wps2, w1n[:, kc, hc * P:(hc + 1) * P], ident)
                nc.vector.tensor_copy(w1T[:, hc, kc * P:(kc + 1) * P], wps2)

        v_sb = singles.tile([1, H], f32, name="v_sb")
        nc.sync.dma_start(out=v_sb, in_=V)
        vT = singles.tile([P, HC], f32r, name="vT")
        hid_sb = singles.tile([BL, H], f32, name="hid_sb")
        nc.sync.dma_start(out=hid_sb, in_=hidden)
        hidT = singles.tile([P, HC, BL], f32, name="hidT")
        for c in range(HC):
            vps = psm.tile([P, 1], f32, name="vps", tag="sps")
            nc.tensor.matmul(vps, v_sb[0:1, c * P:(c + 1) * P], ones11[:], start=True, stop=True)
            nc.vector.tensor_copy(vT[:, c:c + 1], vps)
            hps = psm.tile([P, BL], f32, name="hps", tag="sps")
            nc.tensor.matmul(hps, hid_sb[:, c * P:(c + 1) * P], ident[0:BL, 0:BL], start=True, stop=True)
            nc.vector.tensor_copy(hidT[:, c, :], hps)

        # h1T[k, b] = sum_h W1[k, h] * hidden[b, h], laid out [kp, kh, b]
        h1T = singles.tile([P, HC, BL], f32, name="h1T")
        for kh in range(HC):
            h1ps = psm.tile([P, BL], f32, name="h1ps", tag="sps")
            for c in range(HC):
                nc.tensor.matmul(h1ps, w1T[:, c, kh * P:(kh + 1) * P], hidT[:, c, :],
                                 start=(c == 0), stop=(c == HC - 1))
            nc.vector.tensor_copy(h1T[:, kh, :], h1ps)

        ctx_tr = ctx_tiled.rearrange("b (c p) s -> b c p s", p=P)
        attn_r = attn_out.rearrange("b (p t) -> b p t", t=NT)

        # s-ordered exp row, shared across batches (WAR dep on the bounce DMA)
        exp_row = singles.tile([1, S], f32, name="exp_row")
        # view [o, it, j, p]: position = p*64 + it*4 + j  (= s)
        exp_rv = exp_row.rearrange("o (p i j) -> o i j p", i=NST, j=SUB)

        for b in range(BL):
            # ---- score phase ----
            for it in range(NST):
                encT = []
                for c in range(HC):
                    eps = pet.tile([P, ST], f32r, name="eps", tag="eps")
                    for j in range(SUB):
                        t = it * SUB + j
                        nc.tensor.transpose(eps[:, j * P:(j + 1) * P],
                                            enc_sb[b][:, t, c * P:(c + 1) * P], identr)
                    et = work.tile([P, ST], f32r, name=f"encT{c}", tag=f"encT{c}", bufs=3)
                    nc.vector.tensor_copy(et, eps)
                    encT.append(et)
                th = []
                for kh in range(HC):
                    h2ps = ph2.tile([P, ST], f32, name="h2ps", tag="h2ps")
                    for c in range(HC):
                        nc.tensor.matmul(h2ps, w2T[:, c, kh * P:(kh + 1) * P], encT[c],
                                         start=(c == 0), stop=(c == HC - 1))
                    tt = work.tile([P, ST], f32r, name=f"th{kh}", tag=f"th{kh}", bufs=3)
                    nc.scalar.activation(out=tt, in_=h2ps, func=AF.Tanh,
                                         bias=h1T[:, kh, b:b + 1], scale=1.0)
                    th.append(tt)
                sps = psm.tile([1, ST], f32, name="sps", tag="sps")
                for kh in range(HC):
                    nc.tensor.matmul(sps, vT[:, kh:kh + 1], th[kh],
                                     start=(kh == 0), stop=(kh == HC - 1))
                # exp with s-ordered strided write (no max subtraction needed:
                # |score| <= sum|V_h| ~ 8, well inside fp32 exp range)
                nc.scalar.activation(out=exp_rv[0:1, it],
                                     in_=sps[:].rearrange("o (j p) -> o j p", j=SUB),
                                     func=AF.Exp)

            # ---- departition via DRAM bounce (contiguous both ways) ----
            nc.sync.dma_start(out=scr[b], in_=exp_row)
            if b == BL - 1:
                # paced PE activity across the bounce stall so HAM stays at
                # 8/8 for the context matmuls (a >3.4us idle would re-throttle)
                junk = singles.tile([P, P], f32r, name="junk")
                nc.vector.tensor_copy(junk, identr)
                for w in range(24):
                    dps = psm.tile([P, P], f32r, name="dps", tag="sps")
                    nc.tensor.transpose(dps, junk, identr)
                    nc.vector.tensor_copy(junk, dps)
                jout = work.tile([1, 4], f32, name="jout", tag="jout")
                nc.vector.tensor_copy(jout, junk[0:1, 0:4].bitcast(f32))
                nc.gpsimd.dma_start(out=dbg[0:1, 8:12], in_=jout)
            expT = work.tile([P, NT], f32r, name="expT", tag=f"expT{b}", bufs=1)
            # SWDGE cast f32 -> f32r on the way back in
            nc.gpsimd.dma_start(out=expT, in_=scr[b].rearrange("o (p t) -> (o p) t", t=NT))

            # ---- normalization terms ----
            sm = work.tile([P, 1], f32, name="sm", tag="sm")
            nc.vector.reduce_sum(sm, expT.bitcast(f32), axis=AX.X)
            gsm = work.tile([P, 1], f32, name="gsm", tag="gsm")
            nc.gpsimd.partition_all_reduce(gsm, sm, channels=P, reduce_op=bass_isa.ReduceOp.add)
            inv = work.tile([P, 1], f32, name="inv", tag="inv")
            nc.vector.reciprocal(inv, gsm)
            attnN = work.tile([P, NT], f32, name="attnN", tag=f"attnN{b}", bufs=1)
            nc.vector.tensor_scalar_mul(attnN, expT.bitcast(f32), inv[:, 0:1])
            nc.gpsimd.dma_start(out=attn_r[b], in_=attnN)

            # ---- context = (exp @ enc) / Z ----
            cps = psm.tile([1, H], f32, name="cps", tag="cps", bufs=1)
            for t in range(NT):
                nc.tensor.matmul(cps, expT[:, t:t + 1], enc_sb[b][:, t, :],
                                 start=(t == 0), stop=(t == NT - 1))
            ctx_row = work.tile([1, H], f32, name="ctx_row", tag="ctx_row")
            nc.scalar.activation(out=ctx_row, in_=cps, func=AF.Copy,
                                 scale=inv[0:1, 0:1])
            nc.gpsimd.dma_start(out=ctx_out[b:b + 1, :], in_=ctx_row)
            ctxT = work.tile([P, HC], f32, name="ctxT", tag="ctxT")
            for c in range(HC):
                tps = psm.tile([P, 1], f32, name="tps", tag="sps")
                nc.tensor.matmul(tps, ctx_row[0:1, c * P:(c + 1) * P], ones11[:],
                                 start=True, stop=True)
                nc.vector.tensor_copy(ctxT[:, c:c + 1], tps)

            # ---- context_tiled broadcast: seed + repeat-source DMA ----
            if b == 0:
                # w2n is dead after setup; reuse it as the zero source
                zeros = w2n.rearrange("p a b -> p (a b)")[:, 0:SEEDW]
                nc.vector.memset(zeros, 0.0)
            for c in range(HC):
                seed = work.tile([P, SEEDW], f32, name="seed", tag="seed")
                nc.vector.tensor_scalar_add(seed, zeros, ctxT[:, c:c + 1])
                rep = bass.AP(tensor=seed.tensor, offset=seed.offset,
                              ap=[seed.ap[0], [0, S // SEEDW], seed.ap[1]])
                nc.sync.dma_start(
                    out=ctx_tr[b, c].rearrange("p (r w) -> p r w", w=SEEDW),
                    in_=rep)

    nc.compile()
    return nc


def _get_nc():
    if "nc" not in _CACHE:
        _CACHE["nc"] = _build()
    return _CACHE["nc"]


def kernel(hidden, encoder_outputs, W1, W2, V):
    hidden = np.ascontiguousarray(np.asarray(hidden, dtype=np.float32))
    enc = np.ascontiguousarray(np.asarray(encoder_outputs, dtype=np.float32))
    W1 = np.ascontiguousarray(np.asarray(W1, dtype=np.float32))
    W2 = np.ascontiguousarray(np.asarray(W2, dtype=np.float32))
    V = np.ascontiguousarray(np.asarray(V, dtype=np.float32)).reshape(1, H)

    nc = _get_nc()
    in_maps = []
    for i in range(NCORES):
        sl = slice(i * BL, (i + 1) * BL)
        in_maps.append({
            "hidden": hidden[sl],
            "enc": enc[sl],
            "W1": W1,
            "W2": W2,
            "V": V,
        })
    res = bass_utils.run_bass_kernel_spmd(nc, in_maps, core_ids=list(range(NCORES)))
    ctx_tiled = np.concatenate([r["ctx_tiled"] for r in res.results], axis=0)
    ctx = np.concatenate([r["ctx"] for r in res.results], axis=0)
    attn = np.concatenate([r["attn"] for r in res.results], axis=0)
    return ctx_tiled, ctx, attn


# revision 13
# speedup vs baseline: 1.0962x; 1.0858x over previous
"""Bahdanau attention kernel for Trainium2, data-parallel over batch on 8 cores.

Per core (B_local=2, S=8192, H=256), fp32 I/O with fp32r matmuls:
  1. Cache encoder_outputs in SBUF as f32r (cast during DMA), layout
     [128, 64, 256] with s = p*64 + t (contiguous 64KB per partition).
  2. Per 512-s score tile: PE-transpose enc -> encT [h, s], h2 = W2 @ encT
     (PSUM accum over h chunks), tanh(h2 + h1) on ScalarE (bias = h1 col),
     scores = V . tanh via PE with the 1-column V stationary.
  3. exp(scores) written s-ordered into a [1, 8192] row via a strided
     ScalarE output AP (max subtraction is skipped: |score| <= sum|V| ~ 8,
     safely inside fp32 exp range), then departitioned to [128, 64] by a
     DRAM round-trip (contiguous descriptors both ways).
  4. Normalization folded into the epilogue: Z from the transposed exps,
     context = (exp @ enc) * (1/Z) via 64 accumulating matmuls + scaled copy.
  5. context_tiled = broadcast of context over S: one [128, 512] seed per
     h-chunk, DMAed with a repeat-source access pattern (16x) to HBM.
"""

import numpy as np
from contextlib import ExitStack

import concourse.bass as bass
import concourse.bacc as bacc
import concourse.tile as tile
import concourse.bass_isa as bass_isa
import concourse.bass_utils as bass_utils
from concourse import mybir
from concourse.masks import make_identity

f32 = mybir.dt.float32
f32r = mybir.dt.float32r
AF = mybir.ActivationFunctionType
AX = mybir.AxisListType

B, S, H = 16, 8192, 256
NCORES = 8
BL = B // NCORES          # 2 batches per core
P = 128
NT = S // P               # 64 natural tiles per batch (t axis), s = p*64 + t
ST = 512                  # s elements per score tile
NST = S // ST             # 16 score tiles per batch
SUB = ST // P             # 4 psum sub-tiles per score tile
HC = H // P               # 2 h chunks
SEEDW = 512               # broadcast seed width

_CACHE = {}


def _build():
    nc = bacc.Bacc("TRN2", target_bir_lowering=False, debug=False)
    hidden = nc.dram_tensor("hidden", [BL, H], f32, kind="ExternalInput").ap()
    enc = nc.dram_tensor("enc", [BL, S, H], f32, kind="ExternalInput").ap()
    W1 = nc.dram_tensor("W1", [H, H], f32, kind="ExternalInput").ap()
    W2 = nc.dram_tensor("W2", [H, H], f32, kind="ExternalInput").ap()
    V = nc.dram_tensor("V", [1, H], f32, kind="ExternalInput").ap()
    ctx_tiled = nc.dram_tensor("ctx_tiled", [BL, H, S], f32, kind="ExternalOutput").ap()
    scr = [nc.dram_tensor(f"scr{b}", [1, S], f32, kind="ExternalOutput").ap()
           for b in range(BL)]
    ctx_out = nc.dram_tensor("ctx", [BL, H], f32, kind="ExternalOutput").ap()
    dbg = nc.dram_tensor("dbg", [1, 64], f32, kind="ExternalOutput").ap()
    attn_out = nc.dram_tensor("attn", [BL, S], f32, kind="ExternalOutput").ap()

    with tile.TileContext(nc) as tc, ExitStack() as ctx:
        singles = ctx.enter_context(tc.tile_pool(name="singles", bufs=1))
        encp = ctx.enter_context(tc.tile_pool(name="encp", bufs=1))
        work = ctx.enter_context(tc.tile_pool(name="work", bufs=2))
        pet = ctx.enter_context(tc.tile_pool(name="pet", bufs=3, space="PSUM"))
        ph2 = ctx.enter_context(tc.tile_pool(name="ph2", bufs=2, space="PSUM"))
        psm = ctx.enter_context(tc.tile_pool(name="psm", bufs=2, space="PSUM"))

        # ---- encoder cache: leading chunk, then identity, then the rest,
        # so the gpsimd engine frees the identity ops early ----
        encr = enc.rearrange("b (p t) h -> b p t h", t=NT)
        enc_sb = [encp.tile([P, NT, H], f32r, name=f"enc_sb{b}") for b in range(BL)]
        nc.gpsimd.dma_start(out=enc_sb[0][:, 0:4, :], in_=encr[0, :, 0:4, :])

        ident = singles.tile([P, P], f32, name="ident")
        make_identity(nc, ident)
        identr = singles.tile([P, P], f32r, name="identr")
        nc.vector.tensor_copy(identr, ident)
        ones11 = singles.tile([1, 1], f32, name="ones11")
        nc.vector.memset(ones11, 1.0)

        chunks = [(4, 16), (16, 32), (32, 48), (48, 64)]
        for b in range(BL):
            for k0, k1 in ([(0, 4)] if b > 0 else []) + chunks:
                nc.gpsimd.dma_start(out=enc_sb[b][:, k0:k1, :],
                                    in_=encr[b, :, k0:k1, :])

        # ---- weights setup: transposes via PE ----
        w2n = singles.tile([P, HC, H], f32, name="w2n")
        nc.sync.dma_start(out=w2n, in_=W2.rearrange("(kc kp) h -> kp kc h", kp=P))
        w2T = singles.tile([P, HC, H], f32r, name="w2T")
        w1n = singles.tile([P, HC, H], f32, name="w1n")
        nc.sync.dma_start(out=w1n, in_=W1.rearrange("(kc kp) h -> kp kc h", kp=P))
        w1T = singles.tile([P, HC, H], f32, name="w1T")
        for kc in range(HC):
            for hc in range(HC):
                wps = psm.tile([P, P], f32, name="wps", tag="sps")
                nc.tensor.transpose(wps, w2n[:, kc, hc * P:(hc + 1) * P], ident)
                nc.vector.tensor_copy(w2T[:, hc, kc * P:(kc + 1) * P], wps)
                wps2 = psm.tile([P, P], f32, name="wps2", tag="sps")
                nc.tensor.transpose(wps2, w1n[:, kc, hc * P:(hc + 1) * P], ident)
                nc.vector.tensor_copy(w1T[:, hc, kc * P:(kc + 1) * P], wps2)

        v_sb = singles.tile([1, H], f32, name="v_sb")
        nc.sync.dma_start(out=v_sb, in_=V)
        vT = singles.tile([P, HC], f32r, name="vT")
        hid_sb = singles.tile([BL, H], f32, name="hid_sb")
        nc.sync.dma_start(out=hid_sb, in_=hidden)
        hidT = singles.tile([P, HC, BL], f32, name="hidT")
        for c in range(HC):
            vps = psm.tile([P, 1], f32, name="vps", tag="sps")
            nc.tensor.matmul(vps, v_sb[0:1, c * P:(c + 1) * P], ones11[:], start=True, stop=True)
            nc.vector.tensor_copy(vT[:, c:c + 1], vps)
            hps = psm.tile([P, BL], f32, name="hps", tag="sps")
            nc.tensor.matmul(hps, hid_sb[:, c * P:(c + 1) * P], ident[0:BL, 0:BL], start=True, stop=True)
            nc.vector.tensor_copy(hidT[:, c, :], hps)

        # h1T[k, b] = sum_h W1[k, h] * hidden[b, h], laid out [kp, kh, b]
        h1T = singles.tile([P, HC, BL], f32, name="h1T")
        for kh in range(HC):
            h1ps = psm.tile([P, BL], f32, name="h1ps", tag="sps")
            for c in range(HC):
                nc.tensor.matmul(h1ps, w1T[:, c, kh * P:(kh + 1) * P], hidT[:, c, :],
                                 start=(c == 0), stop=(c == HC - 1))
            nc.vector.tensor_copy(h1T[:, kh, :], h1ps)

        ctx_tr = ctx_tiled.rearrange("b (c p) s -> b c p s", p=P)
        attn_r = attn_out.rearrange("b (p t) -> b p t", t=NT)

        # s-ordered exp row, shared across batches (WAR dep on the bounce DMA)
        exp_row = singles.tile([1, S], f32, name="exp_row")
        # view [o, it, j, p]: position = p*64 + it*4 + j  (= s)
        exp_rv = exp_row.rearrange("o (p i j) -> o i j p", i=NST, j=SUB)

        for b in range(BL):
            # ---- score phase ----
            for it in range(NST):
                encT = []
                for c in range(HC):
                    eps = pet.tile([P, ST], f32r, name="eps", tag="eps")
                    for j in range(SUB):
                        t = it * SUB + j
                        nc.tensor.transpose(eps[:, j * P:(j + 1) * P],
                                            enc_sb[b][:, t, c * P:(c + 1) * P], identr)
                    et = work.tile([P, ST], f32r, name=f"encT{c}", tag=f"encT{c}", bufs=3)
                    nc.vector.tensor_copy(et, eps)
                    encT.append(et)
                th = []
                for kh in range(HC):
                    h2ps = ph2.tile([P, ST], f32, name="h2ps", tag="h2ps")
                    for c in range(HC):
                        nc.tensor.matmul(h2ps, w2T[:, c, kh * P:(kh + 1) * P], encT[c],
                                         start=(c == 0), stop=(c == HC - 1))
                    tt = work.tile([P, ST], f32r, name=f"th{kh}", tag=f"th{kh}", bufs=3)
                    nc.scalar.activation(out=tt, in_=h2ps, func=AF.Tanh,
                                         bias=h1T[:, kh, b:b + 1], scale=1.0)
                    th.append(tt)
                sps = psm.tile([1, ST], f32, name="sps", tag="sps")
                for kh in range(HC):
                    nc.tensor.matmul(sps, vT[:, kh:kh + 1], th[kh],
                                     start=(kh == 0), stop=(kh == HC - 1))
                # exp with s-ordered strided write (no max subtraction needed:
                # |score| <= sum|V_h| ~ 8, well inside fp32 exp range)
                nc.scalar.activation(out=exp_rv[0:1, it],
                                     in_=sps[:].rearrange("o (j p) -> o j p", j=SUB),
                                     func=AF.Exp)

            # ---- departition via DRAM bounce (contiguous both ways) ----
            nc.gpsimd.dma_start(out=scr[b], in_=exp_row)
            if b == BL - 1:
                # paced PE activity across the bounce stall so HAM stays at
                # 8/8 for the context matmuls (a >3.4us idle would re-throttle)
                junk = singles.tile([P, P], f32r, name="junk")
                nc.vector.tensor_copy(junk, identr)
                for w in range(8):
                    dps = psm.tile([P, P], f32r, name="dps", tag="sps")
                    nc.tensor.transpose(dps, junk, identr)
                    nc.vector.tensor_copy(junk, dps)
                jout = work.tile([1, 4], f32, name="jout", tag="jout")
                nc.vector.tensor_copy(jout, junk[0:1, 0:4].bitcast(f32))
                nc.gpsimd.dma_start(out=dbg[0:1, 8:12], in_=jout)
            expT = work.tile([P, NT], f32r, name="expT", tag=f"expT{b}", bufs=1)
            # SWDGE cast f32 -> f32r on the way back in
            nc.gpsimd.dma_start(out=expT, in_=scr[b].rearrange("o (p t) -> (o p) t", t=NT))

            # ---- normalization terms ----
            sm = work.tile([P, 1], f32, name="sm", tag="sm")
            nc.vector.reduce_sum(sm, expT.bitcast(f32), axis=AX.X)
            gsm = work.tile([P, 1], f32, name="gsm", tag="gsm")
            nc.gpsimd.partition_all_reduce(gsm, sm, channels=P, reduce_op=bass_isa.ReduceOp.add)
            inv = work.tile([P, 1], f32, name="inv", tag="inv")
            nc.vector.reciprocal(inv, gsm)
            attnN = work.tile([P, NT], f32, name="attnN", tag=f"attnN{b}", bufs=1)
            nc.vector.tensor_scalar_mul(attnN, expT.bitcast(f32), inv[:, 0:1])
            nc.gpsimd.dma_start(out=attn_r[b], in_=attnN)

            # ---- context = (exp @ enc) / Z ----
            cps = psm.tile([1, H], f32, name="cps", tag="cps", bufs=1)
            for t in range(NT):
                nc.tensor.matmul(cps, expT[:, t:t + 1], enc_sb[b][:, t, :],
                                 start=(t == 0), stop=(t == NT - 1))
            ctx_row = work.tile([1, H], f32, name="ctx_row", tag="ctx_row")
            nc.scalar.activation(out=ctx_row, in_=cps, func=AF.Copy,
                                 scale=inv[0:1, 0:1])
            nc.gpsimd.dma_start(out=ctx_out[b:b + 1, :], in_=ctx_row)
            ctxT = work.tile([P, HC], f32, name="ctxT", tag="ctxT")
            for c in range(HC):
                tps = psm.tile([P, 1], f32, name="tps", tag="sps")
                nc.tensor.matmul(tps, ctx_row[0:1, c * P:(c + 1) * P], ones11[:],
                                 start=True, stop=True)
                nc.vector.tensor_copy(ctxT[:, c:c + 1], tps)

            # ---- context_tiled broadcast: seed + repeat-source DMA ----
            if b == 0:
                # w2n is dead after setup; reuse it as the zero source
                zeros = w2n.rearrange("p a b -> p (a b)")[:, 0:SEEDW]
                nc.vector.memset(zeros, 0.0)
            for c in range(HC):
                seed = work.tile([P, SEEDW], f32, name="seed", tag="seed")
                nc.vector.tensor_scalar_add(seed, zeros, ctxT[:, c:c + 1])
                rep = bass.AP(tensor=seed.tensor, offset=seed.offset,
                              ap=[seed.ap[0], [0, S // SEEDW], seed.ap[1]])
                nc.sync.dma_start(
                    out=ctx_tr[b, c].rearrange("p (r w) -> p r w", w=SEEDW),
                    in_=rep)

    nc.compile()
    return nc


def _get_nc():
    if "nc" not in _CACHE:
        _CACHE["nc"] = _build()
    return _CACHE["nc"]


def kernel(hidden, encoder_outputs, W1, W2, V):
    hidden = np.ascontiguousarray(np.asarray(hidden, dtype=np.float32))
    enc = np.ascontiguousarray(np.asarray(encoder_outputs, dtype=np.float32))
    W1 = np.ascontiguousarray(np.asarray(W1, dtype=np.float32))
    W2 = np.ascontiguousarray(np.asarray(W2, dtype=np.float32))
    V = np.ascontiguousarray(np.asarray(V, dtype=np.float32)).reshape(1, H)

    nc = _get_nc()
    in_maps = []
    for i in range(NCORES):
        sl = slice(i * BL, (i + 1) * BL)
        in_maps.append({
            "hidden": hidden[sl],
            "enc": enc[sl],
            "W1": W1,
            "W2": W2,
            "V": V,
        })
    res = bass_utils.run_bass_kernel_spmd(nc, in_maps, core_ids=list(range(NCORES)))
    ctx_tiled = np.concatenate([r["ctx_tiled"] for r in res.results], axis=0)
    ctx = np.concatenate([r["ctx"] for r in res.results], axis=0)
    attn = np.concatenate([r["attn"] for r in res.results], axis=0)
    return ctx_tiled, ctx, attn


# revision 14
# speedup vs baseline: 1.1473x; 1.0466x over previous
"""Bahdanau attention kernel for Trainium2, data-parallel over batch on 8 cores.

Per core (B_local=2, S=8192, H=256), fp32 I/O with fp32r matmuls:
  1. Cache encoder_outputs in SBUF as f32r (cast during DMA), layout
     [128, 64, 256] with s = p*64 + t (contiguous 64KB per partition).
  2. Per 512-s score tile: PE-transpose enc -> encT [h, s], h2 = W2 @ encT
     (PSUM accum over h chunks), tanh(h2 + h1) on ScalarE (bias = h1 col),
     scores = V . tanh via PE with the 1-column V stationary.
  3. exp(scores) written s-ordered into a [1, 8192] row via a strided
     ScalarE output AP (max subtraction is skipped: |score| <= sum|V| ~ 8,
     safely inside fp32 exp range), then departitioned to [128, 64] by a
     DRAM round-trip (contiguous descriptors both ways).
  4. Normalization folded into the epilogue: Z from the transposed exps,
     context = (exp @ enc) * (1/Z) via 64 accumulating matmuls + scaled copy.
  5. context_tiled = broadcast of context over S: one [128, 512] seed per
     h-chunk, DMAed with a repeat-source access pattern (16x) to HBM.
"""

import numpy as np
from contextlib import ExitStack

import concourse.bass as bass
import concourse.bacc as bacc
import concourse.tile as tile
import concourse.bass_isa as bass_isa
import concourse.bass_utils as bass_utils
from concourse import mybir
from concourse.masks import make_identity

f32 = mybir.dt.float32
f32r = mybir.dt.float32r
AF = mybir.ActivationFunctionType
AX = mybir.AxisListType

B, S, H = 16, 8192, 256
NCORES = 8
BL = B // NCORES          # 2 batches per core
P = 128
NT = S // P               # 64 natural tiles per batch (t axis), s = p*64 + t
ST = 512                  # s elements per score tile
NST = S // ST             # 16 score tiles per batch
SUB = ST // P             # 4 psum sub-tiles per score tile
HC = H // P               # 2 h chunks
SEEDW = 512               # broadcast seed width

_CACHE = {}


def _build():
    nc = bacc.Bacc("TRN2", target_bir_lowering=False, debug=False)
    hidden = nc.dram_tensor("hidden", [BL, H], f32, kind="ExternalInput").ap()
    enc = nc.dram_tensor("enc", [BL, S, H], f32, kind="ExternalInput").ap()
    W1 = nc.dram_tensor("W1", [H, H], f32, kind="ExternalInput").ap()
    W2 = nc.dram_tensor("W2", [H, H], f32, kind="ExternalInput").ap()
    V = nc.dram_tensor("V", [1, H], f32, kind="ExternalInput").ap()
    ctx_tiled = nc.dram_tensor("ctx_tiled", [BL, H, S], f32, kind="ExternalOutput").ap()
    scr = [nc.dram_tensor(f"scr{b}", [1, S], f32, kind="ExternalOutput").ap()
           for b in range(BL)]
    ctx_out = nc.dram_tensor("ctx", [BL, H], f32, kind="ExternalOutput").ap()
    dbg = nc.dram_tensor("dbg", [1, 64], f32, kind="ExternalOutput").ap()
    attn_out = nc.dram_tensor("attn", [BL, S], f32, kind="ExternalOutput").ap()

    with tile.TileContext(nc) as tc, ExitStack() as ctx:
        singles = ctx.enter_context(tc.tile_pool(name="singles", bufs=1))
        encp = ctx.enter_context(tc.tile_pool(name="encp", bufs=1))
        work = ctx.enter_context(tc.tile_pool(name="work", bufs=2))
        pet = ctx.enter_context(tc.tile_pool(name="pet", bufs=3, space="PSUM"))
        ph2 = ctx.enter_context(tc.tile_pool(name="ph2", bufs=2, space="PSUM"))
        psm = ctx.enter_context(tc.tile_pool(name="psm", bufs=2, space="PSUM"))

        # ---- encoder cache: leading chunk, then identity, then the rest,
        # so the gpsimd engine frees the identity ops early ----
        encr = enc.rearrange("b (p t) h -> b p t h", t=NT)
        enc_sb = [encp.tile([P, NT, H], f32r, name=f"enc_sb{b}") for b in range(BL)]
        nc.gpsimd.dma_start(out=enc_sb[0][:, 0:2, :], in_=encr[0, :, 0:2, :])
        nc.gpsimd.dma_start(out=enc_sb[0][:, 2:4, :], in_=encr[0, :, 2:4, :])

        ident = singles.tile([P, P], f32, name="ident")
        make_identity(nc, ident)
        identr = singles.tile([P, P], f32r, name="identr")
        nc.vector.tensor_copy(identr, ident)
        ones11 = singles.tile([1, 1], f32, name="ones11")
        nc.vector.memset(ones11, 1.0)

        chunks = [(4, 16), (16, 32), (32, 48), (48, 64)]
        for b in range(BL):
            for k0, k1 in ([(0, 4)] if b > 0 else []) + chunks:
                nc.gpsimd.dma_start(out=enc_sb[b][:, k0:k1, :],
                                    in_=encr[b, :, k0:k1, :])

        # ---- weights setup: transposes via PE ----
        w2n = singles.tile([P, HC, H], f32, name="w2n")
        nc.sync.dma_start(out=w2n, in_=W2.rearrange("(kc kp) h -> kp kc h", kp=P))
        w2T = singles.tile([P, HC, H], f32r, name="w2T")
        w1n = singles.tile([P, HC, H], f32, name="w1n")
        nc.sync.dma_start(out=w1n, in_=W1.rearrange("(kc kp) h -> kp kc h", kp=P))
        w1T = singles.tile([P, HC, H], f32, name="w1T")
        for kc in range(HC):
            for hc in range(HC):
                wps = psm.tile([P, P], f32, name="wps", tag="sps")
                nc.tensor.transpose(wps, w2n[:, kc, hc * P:(hc + 1) * P], ident)
                nc.vector.tensor_copy(w2T[:, hc, kc * P:(kc + 1) * P], wps)
                wps2 = psm.tile([P, P], f32, name="wps2", tag="sps")
                nc.tensor.transpose(wps2, w1n[:, kc, hc * P:(hc + 1) * P], ident)
                nc.vector.tensor_copy(w1T[:, hc, kc * P:(kc + 1) * P], wps2)

        v_sb = singles.tile([1, H], f32, name="v_sb")
        nc.sync.dma_start(out=v_sb, in_=V)
        vT = singles.tile([P, HC], f32r, name="vT")
        hid_sb = singles.tile([BL, H], f32, name="hid_sb")
        nc.sync.dma_start(out=hid_sb, in_=hidden)
        hidT = singles.tile([P, HC, BL], f32, name="hidT")
        for c in range(HC):
            vps = psm.tile([P, 1], f32, name="vps", tag="sps")
            nc.tensor.matmul(vps, v_sb[0:1, c * P:(c + 1) * P], ones11[:], start=True, stop=True)
            nc.vector.tensor_copy(vT[:, c:c + 1], vps)
            hps = psm.tile([P, BL], f32, name="hps", tag="sps")
            nc.tensor.matmul(hps, hid_sb[:, c * P:(c + 1) * P], ident[0:BL, 0:BL], start=True, stop=True)
            nc.vector.tensor_copy(hidT[:, c, :], hps)

        # h1T[k, b] = sum_h W1[k, h] * hidden[b, h], laid out [kp, kh, b]
        h1T = singles.tile([P, HC, BL], f32, name="h1T")
        for kh in range(HC):
            h1ps = psm.tile([P, BL], f32, name="h1ps", tag="sps")
            for c in range(HC):
                nc.tensor.matmul(h1ps, w1T[:, c, kh * P:(kh + 1) * P], hidT[:, c, :],
                                 start=(c == 0), stop=(c == HC - 1))
            nc.vector.tensor_copy(h1T[:, kh, :], h1ps)

        ctx_tr = ctx_tiled.rearrange("b (c p) s -> b c p s", p=P)
        attn_r = attn_out.rearrange("b (p t) -> b p t", t=NT)

        # s-ordered exp row, shared across batches (WAR dep on the bounce DMA)
        exp_row = singles.tile([1, S], f32, name="exp_row")
        # view [o, it, j, p]: position = p*64 + it*4 + j  (= s)
        exp_rv = exp_row.rearrange("o (p i j) -> o i j p", i=NST, j=SUB)

        for b in range(BL):
            # ---- score phase ----
            for it in range(NST):
                encT = []
                for c in range(HC):
                    eps = pet.tile([P, ST], f32r, name="eps", tag="eps")
                    for j in range(SUB):
                        t = it * SUB + j
                        nc.tensor.transpose(eps[:, j * P:(j + 1) * P],
                                            enc_sb[b][:, t, c * P:(c + 1) * P], identr)
                    et = work.tile([P, ST], f32r, name=f"encT{c}", tag=f"encT{c}", bufs=3)
                    nc.vector.tensor_copy(et, eps)
                    encT.append(et)
                th = []
                for kh in range(HC):
                    h2ps = ph2.tile([P, ST], f32, name="h2ps", tag="h2ps")
                    for c in range(HC):
                        nc.tensor.matmul(h2ps, w2T[:, c, kh * P:(kh + 1) * P], encT[c],
                                         start=(c == 0), stop=(c == HC - 1))
                    tt = work.tile([P, ST], f32r, name=f"th{kh}", tag=f"th{kh}", bufs=3)
                    nc.scalar.activation(out=tt, in_=h2ps, func=AF.Tanh,
                                         bias=h1T[:, kh, b:b + 1], scale=1.0)
                    th.append(tt)
                sps = psm.tile([1, ST], f32, name="sps", tag="sps")
                for kh in range(HC):
                    nc.tensor.matmul(sps, vT[:, kh:kh + 1], th[kh],
                                     start=(kh == 0), stop=(kh == HC - 1))
                # exp with s-ordered strided write (no max subtraction needed:
                # |score| <= sum|V_h| ~ 8, well inside fp32 exp range)
                nc.scalar.activation(out=exp_rv[0:1, it],
                                     in_=sps[:].rearrange("o (j p) -> o j p", j=SUB),
                                     func=AF.Exp)

            # ---- departition via DRAM bounce (contiguous both ways) ----
            nc.sync.dma_start(out=scr[b], in_=exp_row)
            if b == BL - 1:
                # paced PE activity across the bounce stall so HAM stays at
                # 8/8 for the context matmuls (a >3.4us idle would re-throttle)
                junk = singles.tile([P, P], f32r, name="junk")
                nc.vector.tensor_copy(junk, identr)
                for w in range(8):
                    dps = psm.tile([P, P], f32r, name="dps", tag="sps")
                    nc.tensor.transpose(dps, junk, identr)
                    nc.vector.tensor_copy(junk, dps)
                jout = work.tile([1, 4], f32, name="jout", tag="jout")
                nc.vector.tensor_copy(jout, junk[0:1, 0:4].bitcast(f32))
                nc.gpsimd.dma_start(out=dbg[0:1, 8:12], in_=jout)
            expT = work.tile([P, NT], f32r, name="expT", tag=f"expT{b}", bufs=1)
            # SWDGE cast f32 -> f32r on the way back in
            nc.gpsimd.dma_start(out=expT, in_=scr[b].rearrange("o (p t) -> (o p) t", t=NT))

            # ---- normalization terms ----
            sm = work.tile([P, 1], f32, name="sm", tag="sm")
            nc.vector.reduce_sum(sm, expT.bitcast(f32), axis=AX.X)
            gsm = work.tile([P, 1], f32, name="gsm", tag="gsm")
            nc.gpsimd.partition_all_reduce(gsm, sm, channels=P, reduce_op=bass_isa.ReduceOp.add)
            inv = work.tile([P, 1], f32, name="inv", tag="inv")
            nc.vector.reciprocal(inv, gsm)
            attnN = work.tile([P, NT], f32, name="attnN", tag=f"attnN{b}", bufs=1)
            nc.vector.tensor_scalar_mul(attnN, expT.bitcast(f32), inv[:, 0:1])
            nc.gpsimd.dma_start(out=attn_r[b], in_=attnN)

            # ---- context = (exp @ enc) / Z ----
            cps = psm.tile([1, H], f32, name="cps", tag="cps", bufs=1)
            for t in range(NT):
                nc.tensor.matmul(cps, expT[:, t:t + 1], enc_sb[b][:, t, :],
                                 start=(t == 0), stop=(t == NT - 1))
            ctx_row = work.tile([1, H], f32, name="ctx_row", tag="ctx_row")
            nc.scalar.activation(out=ctx_row, in_=cps, func=AF.Copy,
                                 scale=inv[0:1, 0:1])
            nc.gpsimd.dma_start(out=ctx_out[b:b + 1, :], in_=ctx_row)
            ctxT = work.tile([P, HC], f32, name="ctxT", tag="ctxT")
            for c in range(HC):
                tps = psm.tile([P, 1], f32, name="tps", tag="sps")
                nc.tensor.matmul(tps, ctx_row[0:1, c * P:(c + 1) * P], ones11[:],
                                 start=True, stop=True)
                nc.vector.tensor_copy(ctxT[:, c:c + 1], tps)

            # ---- context_tiled broadcast: seed + repeat-source DMA ----
            if b == 0:
                # w2n is dead after setup; reuse it as the zero source
                zeros = w2n.rearrange("p a b -> p (a b)")[:, 0:SEEDW]
                nc.vector.memset(zeros, 0.0)
            for c in range(HC):
                seed = work.tile([P, SEEDW], f32, name="seed", tag="seed")
                nc.vector.tensor_scalar_add(seed, zeros, ctxT[:, c:c + 1])
                rep = bass.AP(tensor=seed.tensor, offset=seed.offset,
                              ap=[seed.ap[0], [0, S // SEEDW], seed.ap[1]])
                nc.sync.dma_start(
                    out=ctx_tr[b, c].rearrange("p (r w) -> p r w", w=SEEDW),
                    in_=rep)

    nc.compile()
    return nc


def _get_nc():
    if "nc" not in _CACHE:
        _CACHE["nc"] = _build()
    return _CACHE["nc"]


def kernel(hidden, encoder_outputs, W1, W2, V):
    hidden = np.ascontiguousarray(np.asarray(hidden, dtype=np.float32))
    enc = np.ascontiguousarray(np.asarray(encoder_outputs, dtype=np.float32))
    W1 = np.ascontiguousarray(np.asarray(W1, dtype=np.float32))
    W2 = np.ascontiguousarray(np.asarray(W2, dtype=np.float32))
    V = np.ascontiguousarray(np.asarray(V, dtype=np.float32)).reshape(1, H)

    nc = _get_nc()
    in_maps = []
    for i in range(NCORES):
        sl = slice(i * BL, (i + 1) * BL)
        in_maps.append({
            "hidden": hidden[sl],
            "enc": enc[sl],
            "W1": W1,
            "W2": W2,
            "V": V,
        })
    res = bass_utils.run_bass_kernel_spmd(nc, in_maps, core_ids=list(range(NCORES)))
    ctx_tiled = np.concatenate([r["ctx_tiled"] for r in res.results], axis=0)
    ctx = np.concatenate([r["ctx"] for r in res.results], axis=0)
    attn = np.concatenate([r["attn"] for r in res.results], axis=0)
    return ctx_tiled, ctx, attn
